# revision 33
# baseline (speedup 1.0000x reference)
"""PostCrossAttention Trainium2 kernel.

Reference computation (per batch b):
    qh = (q @ Wq.T)  split into H=8 heads of dh=96   -> [H, N, 96]
    kh = (k @ Wk.T)  likewise
    vh = (v @ Wv.T)  split into H=8 heads of dv=64   -> [H, N, 64]
    S  = qh @ kh.T * SCALE          (SCALE = (256//8)**-0.5 = 32**-0.5)
    A  = softmax(S, axis=-1)
    A  = A * m / (H * sum(m, -1, keepdims))
    x  = A @ vh   -> concat heads -> [N, 512]

Sharding: 8 cores = 4 batches x 2 head-groups (4 heads each).
Each core receives host-pre-transposed fp16 operands and computes the
un-normalized numerator U^T plus the softmax denominator row; the final
division, transpose and head-concat happen on the host.

Device dataflow (per core):
  Projections (PE at full-width M=128): Wq/Wk weights are host-packed
  into 6 blocks of 128 output dims (3 q blocks + 3 k blocks, heads
  concatenated); each block accumulates over the 6 input ci-tiles into
  4x[128,512] PSUM chunks, then is cast back out to per-head qpt/kpt
  tiles (Q casts on ACT, K casts on DVE - both idle during proj).
  Vp natural [tok, dv] as before.

  Attention (per head h, i-half of 1024; phase = 16 j-tiles):
    S.T[j,i] = Kp @ Qp.T  (PE, [128,1024] psum tiles, pool bufs=3)
    e.T = exp(S.T*SCALE - 4ln2)  (ACT, per j-tile; exps of a j-tile pair
      write the two halves of one [128,2,1024] SBUF tile)
    B.T = e.T * masks.T  (DVE, ONE [128,2,1024] op per pair - saves the
      per-op overhead vs two 1024-wide ops)
    U.T[0:64] += Vp[jt].T @ B.T[jt]  (PE, PSUM-accumulated, AV_LAG behind)
    sumexp: pair-adds (+ NQUADS quad-adds) on DVE; ones-matmuls (PE)
      contract the tiles into U.T row 64, emitted PROGRESSIVELY one tile
      behind formation (a deferred block of ones stalled ACT ~2.1us/phase)
    drain U.T -> SBUF (ACT; the attention pacer is ACT at ~18.5us/phase)
      -> DRAM; each phase's leftover A@V + last ones + drain are carried
      into the NEXT phase's jt loop so the boundary never idles ACT.

  DMA schedule (in-flight DMAs on a ring progress round-robin and finish
  TOGETHER, so priority needs staged release):
    sync ring: wq-b0, q0 (4 chunks), wq-rest, q2, q4 | gate(q4) all six
      k tiles | gate(k5) mask planes jt0-3
    scalar ring: wk, q1, q3, q5, then batches dropped into the ACT
      instruction stream at projection block boundaries (delayed issue):
      wv+v after block 2, masks jt4-9 after block 3, jt10-15 after blk 4.
    (The GpSimd software DGE ignores dependency gating - measured - so
     nothing rides it.)

Host: x[i, h*64+d] = U.T[d,i] / (8 * summ[i] * sumexp[i])   (numpy)
"""

import sys

for _p in ("/opt/trn_rl_repo",):
    if _p not in sys.path:
        sys.path.insert(0, _p)

from contextlib import ExitStack

import numpy as np

import concourse.bacc as bacc_mod
import concourse.mybir as mybir
import concourse.tile as tile

F32 = mybir.dt.float32
# PE-only operands (q/k/v/w projections) run fp16 for precision - same PE
# speed as bf16.  Everything the DVE touches (masks, exp, bsb, accumulators,
# vp) must be bf16: DVE's 2x mode engages for bf16 but NOT fp16 (measured
# 653ns vs 1030ns per [128,1024] tensor_tensor).
import ml_dtypes
DT_PE = mybir.dt.float16
DT_PE_NP = np.float16
DT_VE = mybir.dt.bfloat16
DT_VE_NP = ml_dtypes.bfloat16

# Problem constants (hardcoded per harness contract)
B, N, C, CV, H = 4, 2048, 768, 512, 8
DH, DV = C // H, CV // H          # 96, 64
NH = 4                            # heads per core
NDO = NH * DH                     # 384 projected q/k dims per core
NDV = NH * DV                     # 256 projected v dims per core
SCALE = float((256 // 8) ** (-0.5))
# shift exp into comfortable fp16 range; cancels in U/sumexp ratio
EXP_BIAS = float(-4.0 * np.log(2.0))
N_CORES = 8

NQKB = 6                          # packed q/k projection blocks of 128 dims
BW = 128

# ---- tuning knobs ----
import os
AV_LAG = int(os.environ.get("K_AV_LAG", "3"))
NQUADS = int(os.environ.get("K_NQUADS", "4"))
DRAIN_ENG = os.environ.get("K_DRAIN", "act")  # "act" | "dve" | "split"


def build_nc(NT: int = N):
    """Build the per-core Bass program. NT = token count (param for small sims)."""
    NJT = NT // 128               # j tiles
    assert NT % 512 == 0

    NCT = C // 128                # 6 c tiles
    NVT = CV // 128               # 4 cv tiles
    NCH = NT // 512               # 512-col chunks per token range
    WALL = NQKB * NCT * BW + NVT * NDV
    nc = bacc_mod.Bacc()
    # all inputs host-packed to the exact SBUF image: [128, k*W] where
    # partition p row-interleaves rows {p, 128+p, ...} of the logical tensor
    qT = nc.declare_dram_parameter("qT", [128, NCT * NT], DT_PE, isOutput=False)
    kT = nc.declare_dram_parameter("kT", [128, NCT * NT], DT_PE, isOutput=False)
    vT = nc.declare_dram_parameter("vT", [128, NVT * NT], DT_PE, isOutput=False)
    mT = nc.declare_dram_parameter("mT", [128, NJT * NT], DT_VE, isOutput=False)
    wall = nc.declare_dram_parameter("wall", [128, WALL], DT_PE, isOutput=False)
    # out rows 0..63: U^T (numerator, transposed); row 64: sumexp
    out = nc.declare_dram_parameter("out", [DV + 1, NH * NT], F32, isOutput=True)

    IH = min(1024, NT)            # i-half width
    NHF = NT // IH                # number of i-halves
    NPAIR = NJT // 2

    # q/k block -> per-head cast map: block b covers packed dims
    # [128b, 128b+128) of the NDO=384 q (or k) dims; head h owns
    # [96h, 96h+96).  Returns (psum row range, head, head-dim offset),
    # split so every piece satisfies the engine partition-window rule
    # (start 0 -> span<=128, start 64 -> <=64, start 32/96 -> <=32).
    def _pspan(s):
        return {0: 128, 32: 32, 64: 64, 96: 32}[s % 128]

    def cast_map(b):
        out_ = []
        lo, hi = BW * b, BW * b + BW
        for h in range(NH):
            hl, hh = DH * h, DH * h + DH
            a, z = max(lo, hl), min(hi, hh)
            while a < z:
                ln = min(z - a, _pspan(a - lo), _pspan(a - hl))
                out_.append((a - lo, a - lo + ln, h, a - hl))
                a += ln
        return out_

    with ExitStack() as top:
        tc = top.enter_context(tile.TileContext(nc))
        persist = top.enter_context(tc.tile_pool(name="persist", bufs=1))

        # masks (transposed) resident in SBUF
        mt_all = persist.tile([128, NJT, NT], DT_VE, tag="mt", name="mt_all")

        # ---- projections ----
        qpt = [persist.tile([DH, NT], DT_PE, tag=f"qpt{h}", name=f"qpt{h}") for h in range(NH)]
        kpt = [persist.tile([DH, NT], DT_PE, tag=f"kpt{h}", name=f"kpt{h}") for h in range(NH)]
        vp = persist.tile([128, NJT, NDV], DT_VE, tag="vp", name="vp")
        wv_sb = persist.tile([128, NVT * NDV], DT_PE, tag="wv", name="wv_sb")
        gate1 = persist.tile([128, 1], DT_PE, tag="g1", name="gate1")
        gate2 = persist.tile([128, 1], DT_PE, tag="g2", name="gate2")
        gate3 = persist.tile([128, 1], DT_PE, tag="g3", name="gate3")

        v_pool = top.enter_context(tc.tile_pool(name="vtraw", bufs=1))
        with ExitStack() as projctx:
            qkv_pool = projctx.enter_context(tc.tile_pool(name="qkv", bufs=1))
            w_pool = projctx.enter_context(tc.tile_pool(name="w", bufs=1))
            ppsum = projctx.enter_context(
                tc.tile_pool(name="ppsum", bufs=4, space="PSUM"))

            def load_tiles(dram, n_tiles, width, tag, eng, nsplit):
                t = qkv_pool.tile([128, n_tiles, width], DT_PE, tag=tag, name=tag)
                w2 = n_tiles * width
                for s in range(nsplit):
                    a, b2 = s * w2 // nsplit, (s + 1) * w2 // nsplit
                    eng.dma_start(
                        out=t.rearrange("p a n -> p (a n)")[:, a:b2],
                        in_=dram[:, a:b2])
                return t, [t[:, i, :] for i in range(n_tiles)]

            w_sb = w_pool.tile([128, WALL], DT_PE, tag="wall", name="w_sb")
            QK_END = NQKB * NCT * BW
            QB_END = QK_END // 2
            # DMA staging: in-flight DMAs on one ring progress round-robin
            # and complete TOGETHER, so priority requires staged issue.
            # Upfront (phase A, both rings): weights + all q tiles.
            # Later batches are released only as earlier data is consumed:
            # on sync via tiny gate DMAs (SP queue is idle), on scalar by
            # placing the dma_starts later in the ACT instruction stream
            # (delayed issue - the ACT queue reaches them only after the
            # corresponding cast groups, which are paced by the data).
            # sync phase A: q0's weight block first (192KB), then q-even;
            # under round-robin small DMAs complete first, so the first
            # matmul's operands (wq-b0 + q0) land earliest.
            nc.sync.dma_start(out=w_sb[:, 0:NCT * BW], in_=wall[:, 0:NCT * BW])
            nc.scalar.dma_start(out=w_sb[:, QB_END:QK_END],
                                in_=wall[:, QB_END:QK_END])

            qtile = qkv_pool.tile([128, NCT, NT], DT_PE, tag="q", name="q")
            ktile = qkv_pool.tile([128, NCT, NT], DT_PE, tag="k", name="k")
            vtile = v_pool.tile([128, NVT, NT], DT_PE, tag="v", name="v")
            qts = [qtile[:, i, :] for i in range(NCT)]
            kts = [ktile[:, i, :] for i in range(NCT)]
            vts = [vtile[:, i, :] for i in range(NVT)]
            # plain 2D slice APs so subtile deps track reliably (a
            # rearranged-AP write defeats the region matcher - measured:
            # a gate reading such a tile did not wait for its DMA)
            for hh in range(4):
                hq = NT // 4
                nc.sync.dma_start(out=qts[0][:, hh * hq:(hh + 1) * hq],
                                  in_=qT[:, hh * hq:(hh + 1) * hq])
            nc.sync.dma_start(out=w_sb[:, NCT * BW:QB_END],
                              in_=wall[:, NCT * BW:QB_END])
            for ci in (2, 4):
                nc.sync.dma_start(out=qts[ci], in_=qT[:, ci * NT:(ci + 1) * NT])
            for ci in (1, 3, 5):
                nc.scalar.dma_start(out=qts[ci], in_=qT[:, ci * NT:(ci + 1) * NT])
            # sync phase B: ALL k tiles, gated behind sync's own q batch
            # (the scalar route landed k-odd ~12us too late for the
            # ci-ordered K blocks - measured 5.5us PE gap)
            nc.sync.dma_start(out=gate1, in_=qts[NCT - 2][:, NT - 1:NT])
            for ci in range(NCT):
                nc.sync.dma_start(out=kts[ci], in_=kT[:, ci * NT:(ci + 1) * NT])
            # sync phase C: mask planes jt0-3, gated behind k
            nc.sync.dma_start(out=gate2, in_=kts[NCT - 1][:, NT - 1:NT])
            for s in range(2):
                a, b2 = 2 * s, 2 * s + 2
                nc.sync.dma_start(out=mt_all[:, a:b2, :],
                                  in_=mT[:, a * NT:b2 * NT])

            def emit_kodd_dmas():
                pass

            def emit_v_dmas():
                nc.scalar.dma_start(out=wv_sb, in_=wall[:, QK_END:WALL])
                for ci in range(NVT):
                    nc.scalar.dma_start(out=vts[ci],
                                        in_=vT[:, ci * NT:(ci + 1) * NT])

            def emit_mid_mask_dmas():
                for s in range(2, 5):
                    a, b2 = 2 * s, 2 * s + 2
                    nc.scalar.dma_start(out=mt_all[:, a:b2, :],
                                        in_=mT[:, a * NT:b2 * NT])

            def emit_late_mask_dmas():
                for s in range(5, NJT // 2):
                    a, b2 = 2 * s, 2 * s + 2
                    nc.scalar.dma_start(out=mt_all[:, a:b2, :],
                                        in_=mT[:, a * NT:b2 * NT])

            def wqk_lhsT(b, ci):
                o = (b * NCT + ci) * BW
                return w_sb[:, o:o + BW]

            # Q/K projections: 6 packed blocks (3 q then 3 k), ci-outer.
            # Delayed-issue scalar-ring DMA batches are dropped into the ACT
            # stream at block boundaries (see DMA staging note above).
            for blk_i, (t_kind, bb, xts, pts, ceng) in enumerate(
                    [("q", b, qts, qpt, nc.scalar) for b in range(3)]
                    + [("k", b, kts, kpt, nc.vector) for b in range(3)]):
                if blk_i == 2:
                    emit_v_dmas()
                elif blk_i == 3:
                    emit_mid_mask_dmas()
                elif blk_i == 4:
                    emit_late_mask_dmas()
                # psum granularity is 2 banks: pack the 4 chunk tiles as
                # 2 x [128, 2, 512]
                NCP = max(NCH // 2, 1)
                pss2 = [ppsum.tile([BW, min(NCH, 2), 512], F32, tag="pp",
                                   name="pp") for _ in range(NCP)]
                pss = [pss2[ch // 2][:, ch % 2, :] for ch in range(NCH)]
                boff = 0 if t_kind == "q" else 3
                ci_order = [c for c in range(0, NCT, 2)] + \
                           [c for c in range(1, NCT, 2)]
                for cii, ci in enumerate(ci_order):
                    for ch in range(NCH):
                        nc.tensor.matmul(
                            pss[ch],
                            lhsT=wqk_lhsT(boff + bb, ci),
                            rhs=xts[ci][:, ch * 512:(ch + 1) * 512],
                            start=(cii == 0), stop=(cii == NCT - 1),
                            skip_group_check=True,
                        )
                cm = cast_map(bb)
                for ch in range(NCH):
                    for ci_, (p0, p1, h, d0) in enumerate(cm):
                        # alternate casts between ACT and DVE (both idle
                        # during proj) so neither becomes the proj pacer
                        if ch % 2 == 0:
                            nc.scalar.copy(
                                out=pts[h][d0:d0 + (p1 - p0),
                                           ch * 512:(ch + 1) * 512],
                                in_=pss[ch][p0:p1, :])
                        else:
                            nc.vector.tensor_copy(
                                out=pts[h][d0:d0 + (p1 - p0),
                                           ch * 512:(ch + 1) * 512],
                                in_=pss[ch][p0:p1, :])


            # Vp natural: out[tok-tile, dv_all].  Four j-tiles share one
            # "pp" psum slot (4x256 fp32 regions) so ONE 1024-wide cast
            # drains them together - the per-jt cast cadence was measured
            # at ~0.9us/jt and dominated the V-proj phase.
            wvts = [wv_sb[:, i * NDV:(i + 1) * NDV] for i in range(NVT)]
            assert NJT % 4 == 0 or NJT < 4
            GV = 4 if NJT % 4 == 0 else NJT
            for g in range(NJT // GV):
                psw = ppsum.tile([BW, min(NCH, 2), 512], F32, tag="pp",
                                 name="pp")
                flat = psw.rearrange("p a b -> p (a b)")
                for k in range(GV):
                    jt = g * GV + k
                    ps = flat[:, k * NDV:(k + 1) * NDV]
                    for ci in range(NVT):
                        nc.tensor.matmul(
                            ps,
                            lhsT=vts[ci][:, jt * 128:(jt + 1) * 128],
                            rhs=wvts[ci],
                            start=(ci == 0), stop=(ci == NVT - 1),
                            skip_group_check=True,
                        )
                dst = vp[:, g * GV:(g + 1) * GV, :].rearrange("p a b -> p (a b)")
                if g % 2 == 0:
                    nc.scalar.copy(out=dst, in_=flat[:, 0:GV * NDV])
                else:
                    nc.vector.tensor_copy(out=dst, in_=flat[:, 0:GV * NDV])

        # ---- attention ----
        ones = persist.tile([128, 1], DT_VE, tag="ones", name="ones")
        nc.vector.memset(ones, 1.0)
        ebias = persist.tile([128, 1], F32, tag="ebias", name="ebias")
        nc.vector.memset(ebias, EXP_BIAS)

        spsum = top.enter_context(tc.tile_pool(name="spsum", bufs=3, space="PSUM"))
        utpsum = top.enter_context(tc.tile_pool(name="utpsum", bufs=1, space="PSUM"))
        streams = top.enter_context(tc.tile_pool(name="streams", bufs=3))
        accp = top.enter_context(tc.tile_pool(name="accp", bufs=3))
        utsb_pool = top.enter_context(tc.tile_pool(name="utsb", bufs=2))

        pending = []   # deferred drain, flushed early in next phase

        def emit_ones(ut_ps, tiles, qi0, total):
            for k, tl in enumerate(tiles):
                qi = qi0 + k
                for ic in range(IH // 512):
                    sl = slice(ic * 512, (ic + 1) * 512)
                    nc.tensor.matmul(
                        ut_ps[DV:DV + 1, sl],
                        lhsT=ones,
                        rhs=tl[:, sl],
                        start=(qi == 0), stop=(qi == total - 1),
                        skip_group_check=True,
                    )

        def emit_drain(ut_ps, h, half, last=False):
            ut_sb = utsb_pool.tile([DV + 1, IH], F32, tag="utsb", name="ut_sb")
            # ut psum -> SBUF (GPSIMD cannot read PSUM).  The final phase
            # splits the copy across ACT+DVE and the DMA four ways to
            # shorten the kernel tail.
            mode = "split" if last else DRAIN_ENG
            if mode == "act":
                nc.scalar.copy(out=ut_sb, in_=ut_ps[0:DV + 1, :])
            elif mode == "dve":
                nc.vector.tensor_copy(out=ut_sb, in_=ut_ps[0:DV + 1, :])
            else:
                hw2 = IH // 2
                nc.scalar.copy(out=ut_sb[:, 0:hw2], in_=ut_ps[0:DV + 1, 0:hw2])
                nc.vector.tensor_copy(out=ut_sb[:, hw2:IH],
                                      in_=ut_ps[0:DV + 1, hw2:IH])
            o0 = (h * NHF + half) * IH
            if last:
                # parallel dispatch on both HW queues (a sync-only 4-way
                # split serialized ~0.8us per dma_start in the tail)
                hw = IH // 2
                nc.sync.dma_start(out=out[:, o0:o0 + hw], in_=ut_sb[:, 0:hw])
                nc.scalar.dma_start(out=out[:, o0 + hw:o0 + IH],
                                    in_=ut_sb[:, hw:IH])
            else:
                hw = IH // 2
                nc.sync.dma_start(out=out[:, o0:o0 + hw], in_=ut_sb[:, 0:hw])
                nc.sync.dma_start(out=out[:, o0 + hw:o0 + IH],
                                  in_=ut_sb[:, hw:IH])

        # Per phase: 16 j-tiles processed as 8 pairs.  exp(jt) writes one
        # half of a [128,2,IH] pair tile so the mask-mult runs as a single
        # 2048-wide DVE op (saves the per-op overhead).  The exp-sum tree
        # emits its ones-matmuls PROGRESSIVELY as each pair/quad forms, so
        # at the phase boundary only the last tile's ones + the drain
        # remain - this removed a measured ~2.1us/phase ACT stall waiting
        # for a deferred block of ones-matmuls.
        NTILES = 8 - NQUADS            # ones tiles per phase

        def emit_av(ut_ps, h, jt, bsb, sub):
            first, last = (jt == 0), (jt == NJT - 1)
            for ic in range(IH // 512):
                sl = slice(ic * 512, (ic + 1) * 512)
                nc.tensor.matmul(
                    ut_ps[0:DV, sl],
                    lhsT=vp[:, jt, h * DV:(h + 1) * DV],
                    rhs=bsb[:, sub, sl],
                    start=first, stop=last, skip_group_check=True,
                )

        # pending carries the previous phase's tail INTO this phase's jt
        # loop: its leftover A@V matmuls, final ones-matmuls and the drain
        # are interleaved behind the new phase's S stream so the ACT engine
        # never waits at the boundary (measured 2.1-2.8us/phase before).
        for h in range(NH):
            for ihalf in range(NHF):
                i0 = ihalf * IH
                ut_ps = utpsum.tile([128, IH], F32, tag="ut", name="ut")

                pairs = []
                quads = []
                av_queue = []
                ones_ready = []
                ones_n = [0]

                def emit_one(tl, _ut=ut_ps, _n=ones_n):
                    emit_ones(_ut, [tl], _n[0], NTILES)
                    _n[0] += 1

                def flush_pending(jt):
                    if not pending:
                        return
                    tail = pending[0]
                    avs = tail["avs"]
                    for _ in range(4):
                        if avs:
                            emit_av(tail["ut"], tail["h"], *avs.pop(0))
                    if not avs and jt >= 2:
                        for tl in tail["ones"]:
                            tail["emit_one"](tl)
                        emit_drain(tail["ut"], tail["h"], tail["half"])
                        pending.pop(0)

                expp = None
                for jt in range(NJT):
                    if jt % 2 == 0:
                        expp = streams.tile([128, 2, IH], DT_VE, tag="expst",
                                            name="expst", bufs=4)
                    s_ps = spsum.tile([128, IH], F32, tag="s", name="s_ps")
                    for q2 in range(IH // 512):
                        nc.tensor.matmul(
                            s_ps[:, q2 * 512:(q2 + 1) * 512],
                            lhsT=kpt[h][:, jt * 128:(jt + 1) * 128],
                            rhs=qpt[h][:, i0 + q2 * 512: i0 + (q2 + 1) * 512],
                            start=True, stop=True,
                        )
                    nc.scalar.activation(
                        out=expp[:, jt % 2, :], in_=s_ps,
                        func=mybir.ActivationFunctionType.Exp, scale=SCALE,
                        bias=ebias[:, 0:1],
                    )
                    flush_pending(jt)
                    if jt % 2 == 1:
                        bsbp = streams.tile([128, 2, IH], DT_VE, tag="b",
                                            name="bsb",
                                            bufs=AV_LAG // 2 + 5)
                        nc.vector.tensor_tensor(
                            out=bsbp, in0=expp,
                            in1=mt_all[:, jt - 1:jt + 1, i0:i0 + IH],
                            op=mybir.AluOpType.mult)
                        av_queue.append((jt - 1, bsbp, 0))
                        av_queue.append((jt, bsbp, 1))
                        pr = accp.tile([128, IH], DT_VE, tag="pair",
                                       name="pair", bufs=8)
                        nc.vector.tensor_tensor(
                            out=pr, in0=expp[:, 0, :], in1=expp[:, 1, :],
                            op=mybir.AluOpType.add)
                        pairs.append(pr)
                        if len(pairs) <= 2 * NQUADS:
                            if len(pairs) % 2 == 0:
                                quad = accp.tile([128, IH], DT_VE, tag="quad",
                                                 name="quad", bufs=4)
                                nc.vector.tensor_tensor(
                                    out=quad, in0=pairs[-2], in1=pairs[-1],
                                    op=mybir.AluOpType.add)
                                quads.append(quad)
                                ones_ready.append(quad)
                        else:
                            ones_ready.append(pr)
                    if not pending:
                        while len(av_queue) > AV_LAG:
                            emit_av(ut_ps, h, *av_queue.pop(0))
                        # ones-matmuls lag one tile behind formation so the
                        # PE never waits on the DVE tree mid-phase
                        while len(ones_ready) > 1:
                            emit_one(ones_ready.pop(0))

                if h == NH - 1 and ihalf == NHF - 1:
                    for jb in av_queue:
                        emit_av(ut_ps, h, *jb)
                    for tl in ones_ready:
                        emit_one(tl)
                    emit_drain(ut_ps, h, ihalf, last=True)
                else:
                    pending.append({
                        "ut": ut_ps, "h": h, "half": ihalf,
                        "avs": av_queue, "ones": ones_ready,
                        "emit_one": emit_one,
                    })
        if pending:
            tail = pending.pop(0)
            for jb in tail["avs"]:
                emit_av(tail["ut"], tail["h"], *jb)
            for tl in tail["ones"]:
                tail["emit_one"](tl)
            emit_drain(tail["ut"], tail["h"], tail["half"])

    nc.finalize()
    return nc


_NC_CACHE: dict = {}


def get_nc(NT: int = N):
    if NT not in _NC_CACHE:
        _NC_CACHE[NT] = build_nc(NT)
    return _NC_CACHE[NT]


def _pack(x):
    """[k*128, W] -> [128, k*W]: partition p holds rows {p, 128+p, ...}."""
    k = x.shape[0] // 128
    return x.reshape(k, 128, -1).transpose(1, 0, 2).reshape(128, -1)


def pack_core(qb, kb, vb, mb, wq_s, wk_s, wv_s):
    """Build one core's packed input dict from raw (transposed) slices.

    q/k/v/w go fp16 (PE-only); masks go bf16 (DVE operand).
    wq_s/wk_s: [C_in=768, NDO=384] (W.T slices); packed into 6 blocks of
    128 output dims (3 q + 3 k), block-major, ci-minor."""

    def f16(x):
        return np.ascontiguousarray(_pack(x.astype(np.float32).astype(DT_PE_NP)))

    NCT = C // 128
    wqk = np.concatenate([wq_s, wk_s], axis=1).astype(np.float32).astype(DT_PE_NP)
    P = _pack(wqk)                     # [128, NCT*2*NDO]
    blocks = []
    for b in range(NQKB):
        for ci in range(NCT):
            blocks.append(P[:, ci * 2 * NDO + b * BW: ci * 2 * NDO + (b + 1) * BW])
    wall = np.concatenate(
        blocks + [_pack(wv_s.astype(np.float32).astype(DT_PE_NP))], axis=1)
    mbf = np.ascontiguousarray(_pack(mb.astype(np.float32).astype(DT_VE_NP)))
    return {
        "qT": f16(qb), "kT": f16(kb), "vT": f16(vb), "mT": mbf,
        "wall": np.ascontiguousarray(wall),
    }


def make_in_maps(q, k, v, masks, Wq, Wk, Wv):
    """Host-side shard + layout prep. Returns per-core input dicts."""
    in_maps = []
    for c in range(N_CORES):
        b, hg = c // 2, c % 2
        in_maps.append(pack_core(
            q[b].T, k[b].T, v[b].T, masks[b].T,
            Wq[hg * NDO:(hg + 1) * NDO, :].T,
            Wk[hg * NDO:(hg + 1) * NDO, :].T,
            Wv[hg * NDV:(hg + 1) * NDV, :].T,
        ))
    return in_maps


def postprocess(res_out, masks_b, NT=N):
    """Host epilogue for one core: [65, NH*NT] raw -> [NT, NDV] slice.

    res_out rows 0..63: U^T numerator; row 64: sumexp (both carry the
    same exp bias factor, which cancels in the ratio).
    x[i, h*DV+d] = U^T[d, i] / (H * summ[i] * sumexp[i])
    """
    IH = min(1024, NT)
    NHF = NT // IH
    arr = np.asarray(res_out, np.float32).reshape(DV + 1, NH, NHF, IH)
    U = arr[:DV]                       # [DV, NH, NHF, IH]
    se = arr[DV]                       # [NH, NHF, IH]
    summ = masks_b.astype(np.float64).sum(axis=1).astype(np.float32)
    den = float(H) * summ.reshape(NHF, IH)[None, :, :] * se
    xh = U / den[None]                 # [DV, NH, NHF, IH]
    return xh.transpose(2, 3, 1, 0).reshape(NT, NH * DV)


def _reset_device():
    import ctypes
    try:
        lib = ctypes.CDLL("/opt/axon/libaxon_pjrt.so")
        lib.axon_reset.restype = ctypes.c_int64
        lib.axon_reset()
    except Exception:
        pass


def kernel(q, k, v, masks, Wq, Wk, Wv, **_unused):
    from concourse.bass_utils import run_bass_kernel_spmd

    q, k, v, masks = (np.asarray(x) for x in (q, k, v, masks))
    Wq, Wk, Wv = (np.asarray(x) for x in (Wq, Wk, Wv))

    nc = get_nc(N)
    in_maps = make_in_maps(q, k, v, masks, Wq, Wk, Wv)
    try:
        res = run_bass_kernel_spmd(
            nc, in_maps, core_ids=list(range(N_CORES))).results
    except Exception:
        # wedged accelerator (e.g. NRT_EXEC_UNIT_UNRECOVERABLE) - reset + retry
        _reset_device()
        res = run_bass_kernel_spmd(
            nc, in_maps, core_ids=list(range(N_CORES))).results

    full = np.empty((B, N, CV), np.float32)
    for c in range(N_CORES):
        b, hg = c // 2, c % 2
        full[b][:, hg * NDV:(hg + 1) * NDV] = postprocess(
            res[c]["out"], masks[b])
    return full


# revision 34
# speedup vs baseline: 1.0327x; 1.0327x over previous
"""PostCrossAttention Trainium2 kernel.

Reference computation (per batch b):
    qh = (q @ Wq.T)  split into H=8 heads of dh=96   -> [H, N, 96]
    kh = (k @ Wk.T)  likewise
    vh = (v @ Wv.T)  split into H=8 heads of dv=64   -> [H, N, 64]
    S  = qh @ kh.T * SCALE          (SCALE = (256//8)**-0.5 = 32**-0.5)
    A  = softmax(S, axis=-1)
    A  = A * m / (H * sum(m, -1, keepdims))
    x  = A @ vh   -> concat heads -> [N, 512]

Sharding: 8 cores = 4 batches x 2 head-groups (4 heads each).
Each core receives host-pre-transposed fp16 operands and computes the
un-normalized numerator U^T plus the softmax denominator row; the final
division, transpose and head-concat happen on the host.

Device dataflow (per core):
  Projections (PE at full-width M=128): Wq/Wk weights are host-packed
  into 6 blocks of 128 output dims (3 q blocks + 3 k blocks, heads
  concatenated); each block accumulates over the 6 input ci-tiles into
  4x[128,512] PSUM chunks, then is cast back out to per-head qpt/kpt
  tiles (Q casts on ACT, K casts on DVE - both idle during proj).
  Vp natural [tok, dv] as before.

  Attention (per head h, i-half of 1024; phase = 16 j-tiles):
    S.T[j,i] = Kp @ Qp.T  (PE, [128,1024] psum tiles, pool bufs=3)
    e.T = exp(S.T*SCALE - 4ln2)  (ACT, per j-tile; exps of a j-tile pair
      write the two halves of one [128,2,1024] SBUF tile)
    B.T = e.T * masks.T  (DVE, ONE [128,2,1024] op per pair - saves the
      per-op overhead vs two 1024-wide ops)
    U.T[0:64] += Vp[jt].T @ B.T[jt]  (PE, PSUM-accumulated, AV_LAG behind)
    sumexp: pair-adds (+ NQUADS quad-adds) on DVE; ones-matmuls (PE)
      contract the tiles into U.T row 64, emitted PROGRESSIVELY one tile
      behind formation (a deferred block of ones stalled ACT ~2.1us/phase)
    drain U.T -> SBUF (ACT; the attention pacer is ACT at ~18.5us/phase)
      -> DRAM; each phase's leftover A@V + last ones + drain are carried
      into the NEXT phase's jt loop so the boundary never idles ACT.

  DMA schedule (in-flight DMAs on a ring progress round-robin and finish
  TOGETHER, so priority needs staged release):
    sync ring: wq-b0, q0 (4 chunks), wq-rest, q2, q4 | gate(q4) all six
      k tiles | gate(k5) mask planes jt0-3
    scalar ring: wk, q1, q3, q5, then batches dropped into the ACT
      instruction stream at projection block boundaries (delayed issue):
      wv+v after block 2, masks jt4-9 after block 3, jt10-15 after blk 4.
    (The GpSimd software DGE ignores dependency gating - measured - so
     nothing rides it.)

Host: x[i, h*64+d] = U.T[d,i] / (8 * summ[i] * sumexp[i])   (numpy)
"""

import sys

for _p in ("/opt/trn_rl_repo",):
    if _p not in sys.path:
        sys.path.insert(0, _p)

from contextlib import ExitStack

import numpy as np

import concourse.bacc as bacc_mod
import concourse.mybir as mybir
import concourse.tile as tile

F32 = mybir.dt.float32
# PE-only operands (q/k/v/w projections) run fp16 for precision - same PE
# speed as bf16.  Everything the DVE touches (masks, exp, bsb, accumulators,
# vp) must be bf16: DVE's 2x mode engages for bf16 but NOT fp16 (measured
# 653ns vs 1030ns per [128,1024] tensor_tensor).
import ml_dtypes
DT_PE = mybir.dt.float16
DT_PE_NP = np.float16
DT_VE = mybir.dt.bfloat16
DT_VE_NP = ml_dtypes.bfloat16

# Problem constants (hardcoded per harness contract)
B, N, C, CV, H = 4, 2048, 768, 512, 8
DH, DV = C // H, CV // H          # 96, 64
NH = 4                            # heads per core
NDO = NH * DH                     # 384 projected q/k dims per core
NDV = NH * DV                     # 256 projected v dims per core
SCALE = float((256 // 8) ** (-0.5))
# shift exp into comfortable fp16 range; cancels in U/sumexp ratio
EXP_BIAS = float(-4.0 * np.log(2.0))
N_CORES = 8

NQKB = 6                          # packed q/k projection blocks of 128 dims
BW = 128

# ---- tuning knobs ----
import os
AV_LAG = int(os.environ.get("K_AV_LAG", "4"))
NQUADS = int(os.environ.get("K_NQUADS", "4"))
DRAIN_ENG = os.environ.get("K_DRAIN", "act")  # "act" | "dve" | "split"


def build_nc(NT: int = N):
    """Build the per-core Bass program. NT = token count (param for small sims)."""
    NJT = NT // 128               # j tiles
    assert NT % 512 == 0

    NCT = C // 128                # 6 c tiles
    NVT = CV // 128               # 4 cv tiles
    NCH = NT // 512               # 512-col chunks per token range
    WALL = NQKB * NCT * BW + NVT * NDV
    nc = bacc_mod.Bacc()
    # all inputs host-packed to the exact SBUF image: [128, k*W] where
    # partition p row-interleaves rows {p, 128+p, ...} of the logical tensor
    qT = nc.declare_dram_parameter("qT", [128, NCT * NT], DT_PE, isOutput=False)
    kT = nc.declare_dram_parameter("kT", [128, NCT * NT], DT_PE, isOutput=False)
    vT = nc.declare_dram_parameter("vT", [128, NVT * NT], DT_PE, isOutput=False)
    mT = nc.declare_dram_parameter("mT", [128, NJT * NT], DT_VE, isOutput=False)
    wall = nc.declare_dram_parameter("wall", [128, WALL], DT_PE, isOutput=False)
    # out rows 0..63: U^T (numerator, transposed); row 64: sumexp
    out = nc.declare_dram_parameter("out", [DV + 1, NH * NT], F32, isOutput=True)

    IH = min(1024, NT)            # i-half width
    NHF = NT // IH                # number of i-halves
    NPAIR = NJT // 2

    # q/k block -> per-head cast map: block b covers packed dims
    # [128b, 128b+128) of the NDO=384 q (or k) dims; head h owns
    # [96h, 96h+96).  Returns (psum row range, head, head-dim offset),
    # split so every piece satisfies the engine partition-window rule
    # (start 0 -> span<=128, start 64 -> <=64, start 32/96 -> <=32).
    def _pspan(s):
        return {0: 128, 32: 32, 64: 64, 96: 32}[s % 128]

    def cast_map(b):
        out_ = []
        lo, hi = BW * b, BW * b + BW
        for h in range(NH):
            hl, hh = DH * h, DH * h + DH
            a, z = max(lo, hl), min(hi, hh)
            while a < z:
                ln = min(z - a, _pspan(a - lo), _pspan(a - hl))
                out_.append((a - lo, a - lo + ln, h, a - hl))
                a += ln
        return out_

    with ExitStack() as top:
        tc = top.enter_context(tile.TileContext(nc))
        persist = top.enter_context(tc.tile_pool(name="persist", bufs=1))

        # masks (transposed) resident in SBUF
        mt_all = persist.tile([128, NJT, NT], DT_VE, tag="mt", name="mt_all")

        # ---- projections ----
        qpt = [persist.tile([DH, NT], DT_PE, tag=f"qpt{h}", name=f"qpt{h}") for h in range(NH)]
        kpt = [persist.tile([DH, NT], DT_PE, tag=f"kpt{h}", name=f"kpt{h}") for h in range(NH)]
        vp = persist.tile([128, NJT, NDV], DT_VE, tag="vp", name="vp")
        wv_sb = persist.tile([128, NVT * NDV], DT_PE, tag="wv", name="wv_sb")
        gate1 = persist.tile([128, 1], DT_PE, tag="g1", name="gate1")
        gate2 = persist.tile([128, 1], DT_PE, tag="g2", name="gate2")
        gate3 = persist.tile([128, 1], DT_PE, tag="g3", name="gate3")

        v_pool = top.enter_context(tc.tile_pool(name="vtraw", bufs=1))
        with ExitStack() as projctx:
            qkv_pool = projctx.enter_context(tc.tile_pool(name="qkv", bufs=1))
            w_pool = projctx.enter_context(tc.tile_pool(name="w", bufs=1))
            ppsum = projctx.enter_context(
                tc.tile_pool(name="ppsum", bufs=4, space="PSUM"))

            def load_tiles(dram, n_tiles, width, tag, eng, nsplit):
                t = qkv_pool.tile([128, n_tiles, width], DT_PE, tag=tag, name=tag)
                w2 = n_tiles * width
                for s in range(nsplit):
                    a, b2 = s * w2 // nsplit, (s + 1) * w2 // nsplit
                    eng.dma_start(
                        out=t.rearrange("p a n -> p (a n)")[:, a:b2],
                        in_=dram[:, a:b2])
                return t, [t[:, i, :] for i in range(n_tiles)]

            w_sb = w_pool.tile([128, WALL], DT_PE, tag="wall", name="w_sb")
            QK_END = NQKB * NCT * BW
            QB_END = QK_END // 2
            # DMA staging: in-flight DMAs on one ring progress round-robin
            # and complete TOGETHER, so priority requires staged issue.
            # Upfront (phase A, both rings): weights + all q tiles.
            # Later batches are released only as earlier data is consumed:
            # on sync via tiny gate DMAs (SP queue is idle), on scalar by
            # placing the dma_starts later in the ACT instruction stream
            # (delayed issue - the ACT queue reaches them only after the
            # corresponding cast groups, which are paced by the data).
            # sync phase A: q0's weight block first (192KB), then q-even;
            # under round-robin small DMAs complete first, so the first
            # matmul's operands (wq-b0 + q0) land earliest.
            nc.sync.dma_start(out=w_sb[:, 0:NCT * BW], in_=wall[:, 0:NCT * BW])
            nc.scalar.dma_start(out=w_sb[:, QB_END:QK_END],
                                in_=wall[:, QB_END:QK_END])

            qtile = qkv_pool.tile([128, NCT, NT], DT_PE, tag="q", name="q")
            ktile = qkv_pool.tile([128, NCT, NT], DT_PE, tag="k", name="k")
            vtile = v_pool.tile([128, NVT, NT], DT_PE, tag="v", name="v")
            qts = [qtile[:, i, :] for i in range(NCT)]
            kts = [ktile[:, i, :] for i in range(NCT)]
            vts = [vtile[:, i, :] for i in range(NVT)]
            # plain 2D slice APs so subtile deps track reliably (a
            # rearranged-AP write defeats the region matcher - measured:
            # a gate reading such a tile did not wait for its DMA)
            for hh in range(4):
                hq = NT // 4
                nc.sync.dma_start(out=qts[0][:, hh * hq:(hh + 1) * hq],
                                  in_=qT[:, hh * hq:(hh + 1) * hq])
            nc.sync.dma_start(out=w_sb[:, NCT * BW:QB_END],
                              in_=wall[:, NCT * BW:QB_END])
            for ci in (2, 4):
                nc.sync.dma_start(out=qts[ci], in_=qT[:, ci * NT:(ci + 1) * NT])
            for ci in (1, 3, 5):
                nc.scalar.dma_start(out=qts[ci], in_=qT[:, ci * NT:(ci + 1) * NT])
            # sync phase B: ALL k tiles, gated behind sync's own q batch
            # (the scalar route landed k-odd ~12us too late for the
            # ci-ordered K blocks - measured 5.5us PE gap)
            nc.sync.dma_start(out=gate1, in_=qts[NCT - 2][:, NT - 1:NT])
            for ci in range(NCT):
                nc.sync.dma_start(out=kts[ci], in_=kT[:, ci * NT:(ci + 1) * NT])
            # sync phase C: mask planes jt0-3, gated behind k
            nc.sync.dma_start(out=gate2, in_=kts[NCT - 1][:, NT - 1:NT])
            for s in range(2):
                a, b2 = 2 * s, 2 * s + 2
                nc.sync.dma_start(out=mt_all[:, a:b2, :],
                                  in_=mT[:, a * NT:b2 * NT])

            def emit_kodd_dmas():
                pass

            def emit_v_dmas():
                nc.scalar.dma_start(out=wv_sb, in_=wall[:, QK_END:WALL])
                for ci in range(NVT):
                    nc.scalar.dma_start(out=vts[ci],
                                        in_=vT[:, ci * NT:(ci + 1) * NT])

            def emit_mid_mask_dmas():
                for s in range(2, 5):
                    a, b2 = 2 * s, 2 * s + 2
                    nc.scalar.dma_start(out=mt_all[:, a:b2, :],
                                        in_=mT[:, a * NT:b2 * NT])

            def emit_late_mask_dmas():
                for s in range(5, NJT // 2):
                    a, b2 = 2 * s, 2 * s + 2
                    nc.scalar.dma_start(out=mt_all[:, a:b2, :],
                                        in_=mT[:, a * NT:b2 * NT])

            def wqk_lhsT(b, ci):
                o = (b * NCT + ci) * BW
                return w_sb[:, o:o + BW]

            # Q/K projections: 6 packed blocks (3 q then 3 k), ci-outer.
            # Delayed-issue scalar-ring DMA batches are dropped into the ACT
            # stream at block boundaries (see DMA staging note above).
            for blk_i, (t_kind, bb, xts, pts, ceng) in enumerate(
                    [("q", b, qts, qpt, nc.scalar) for b in range(3)]
                    + [("k", b, kts, kpt, nc.vector) for b in range(3)]):
                if blk_i == 2:
                    emit_v_dmas()
                elif blk_i == 3:
                    emit_mid_mask_dmas()
                elif blk_i == 4:
                    emit_late_mask_dmas()
                # psum granularity is 2 banks: pack the 4 chunk tiles as
                # 2 x [128, 2, 512]
                NCP = max(NCH // 2, 1)
                pss2 = [ppsum.tile([BW, min(NCH, 2), 512], F32, tag="pp",
                                   name="pp") for _ in range(NCP)]
                pss = [pss2[ch // 2][:, ch % 2, :] for ch in range(NCH)]
                boff = 0 if t_kind == "q" else 3
                ci_order = [c for c in range(0, NCT, 2)] + \
                           [c for c in range(1, NCT, 2)]
                for cii, ci in enumerate(ci_order):
                    for ch in range(NCH):
                        nc.tensor.matmul(
                            pss[ch],
                            lhsT=wqk_lhsT(boff + bb, ci),
                            rhs=xts[ci][:, ch * 512:(ch + 1) * 512],
                            start=(cii == 0), stop=(cii == NCT - 1),
                            skip_group_check=True,
                        )
                cm = cast_map(bb)
                for ch in range(NCH):
                    for ci_, (p0, p1, h, d0) in enumerate(cm):
                        # alternate casts between ACT and DVE (both idle
                        # during proj) so neither becomes the proj pacer
                        if ch % 2 == 0:
                            nc.scalar.copy(
                                out=pts[h][d0:d0 + (p1 - p0),
                                           ch * 512:(ch + 1) * 512],
                                in_=pss[ch][p0:p1, :])
                        else:
                            nc.vector.tensor_copy(
                                out=pts[h][d0:d0 + (p1 - p0),
                                           ch * 512:(ch + 1) * 512],
                                in_=pss[ch][p0:p1, :])


            # Vp natural: out[tok-tile, dv_all].  Four j-tiles share one
            # "pp" psum slot (4x256 fp32 regions) so ONE 1024-wide cast
            # drains them together - the per-jt cast cadence was measured
            # at ~0.9us/jt and dominated the V-proj phase.
            wvts = [wv_sb[:, i * NDV:(i + 1) * NDV] for i in range(NVT)]
            assert NJT % 4 == 0 or NJT < 4
            GV = 4 if NJT % 4 == 0 else NJT
            for g in range(NJT // GV):
                psw = ppsum.tile([BW, min(NCH, 2), 512], F32, tag="pp",
                                 name="pp")
                flat = psw.rearrange("p a b -> p (a b)")
                for k in range(GV):
                    jt = g * GV + k
                    ps = flat[:, k * NDV:(k + 1) * NDV]
                    for ci in range(NVT):
                        nc.tensor.matmul(
                            ps,
                            lhsT=vts[ci][:, jt * 128:(jt + 1) * 128],
                            rhs=wvts[ci],
                            start=(ci == 0), stop=(ci == NVT - 1),
                            skip_group_check=True,
                        )
                dst = vp[:, g * GV:(g + 1) * GV, :].rearrange("p a b -> p (a b)")
                if g % 2 == 0:
                    nc.scalar.copy(out=dst, in_=flat[:, 0:GV * NDV])
                else:
                    nc.vector.tensor_copy(out=dst, in_=flat[:, 0:GV * NDV])

        # ---- attention ----
        ones = persist.tile([128, 1], DT_VE, tag="ones", name="ones")
        nc.vector.memset(ones, 1.0)
        ebias = persist.tile([128, 1], F32, tag="ebias", name="ebias")
        nc.vector.memset(ebias, EXP_BIAS)

        spsum = top.enter_context(tc.tile_pool(name="spsum", bufs=3, space="PSUM"))
        utpsum = top.enter_context(tc.tile_pool(name="utpsum", bufs=1, space="PSUM"))
        streams = top.enter_context(tc.tile_pool(name="streams", bufs=3))
        accp = top.enter_context(tc.tile_pool(name="accp", bufs=3))
        utsb_pool = top.enter_context(tc.tile_pool(name="utsb", bufs=2))

        pending = []   # deferred drain, flushed early in next phase

        def emit_ones(ut_ps, tiles, qi0, total):
            for k, tl in enumerate(tiles):
                qi = qi0 + k
                for ic in range(IH // 512):
                    sl = slice(ic * 512, (ic + 1) * 512)
                    nc.tensor.matmul(
                        ut_ps[DV:DV + 1, sl],
                        lhsT=ones,
                        rhs=tl[:, sl],
                        start=(qi == 0), stop=(qi == total - 1),
                        skip_group_check=True,
                    )

        def emit_drain(ut_ps, h, half, last=False):
            ut_sb = utsb_pool.tile([DV + 1, IH], F32, tag="utsb", name="ut_sb")
            # ut psum -> SBUF (GPSIMD cannot read PSUM).  The final phase
            # splits the copy across ACT+DVE and the DMA four ways to
            # shorten the kernel tail.
            mode = "split" if last else DRAIN_ENG
            if mode == "act":
                nc.scalar.copy(out=ut_sb, in_=ut_ps[0:DV + 1, :])
            elif mode == "dve":
                nc.vector.tensor_copy(out=ut_sb, in_=ut_ps[0:DV + 1, :])
            else:
                hw2 = IH // 2
                nc.scalar.copy(out=ut_sb[:, 0:hw2], in_=ut_ps[0:DV + 1, 0:hw2])
                nc.vector.tensor_copy(out=ut_sb[:, hw2:IH],
                                      in_=ut_ps[0:DV + 1, hw2:IH])
            o0 = (h * NHF + half) * IH
            if last:
                # parallel dispatch on both HW queues (a sync-only 4-way
                # split serialized ~0.8us per dma_start in the tail)
                hw = IH // 2
                nc.sync.dma_start(out=out[:, o0:o0 + hw], in_=ut_sb[:, 0:hw])
                nc.scalar.dma_start(out=out[:, o0 + hw:o0 + IH],
                                    in_=ut_sb[:, hw:IH])
            else:
                hw = IH // 2
                nc.sync.dma_start(out=out[:, o0:o0 + hw], in_=ut_sb[:, 0:hw])
                nc.sync.dma_start(out=out[:, o0 + hw:o0 + IH],
                                  in_=ut_sb[:, hw:IH])

        # Per phase: 16 j-tiles processed as 8 pairs.  exp(jt) writes one
        # half of a [128,2,IH] pair tile so the mask-mult runs as a single
        # 2048-wide DVE op (saves the per-op overhead).  The exp-sum tree
        # emits its ones-matmuls PROGRESSIVELY as each pair/quad forms, so
        # at the phase boundary only the last tile's ones + the drain
        # remain - this removed a measured ~2.1us/phase ACT stall waiting
        # for a deferred block of ones-matmuls.
        NTILES = 8 - NQUADS            # ones tiles per phase

        def emit_av(ut_ps, h, jt, bsb, sub):
            first, last = (jt == 0), (jt == NJT - 1)
            for ic in range(IH // 512):
                sl = slice(ic * 512, (ic + 1) * 512)
                nc.tensor.matmul(
                    ut_ps[0:DV, sl],
                    lhsT=vp[:, jt, h * DV:(h + 1) * DV],
                    rhs=bsb[:, sub, sl],
                    start=first, stop=last, skip_group_check=True,
                )

        # pending carries the previous phase's tail INTO this phase's jt
        # loop: its leftover A@V matmuls, final ones-matmuls and the drain
        # are interleaved behind the new phase's S stream so the ACT engine
        # never waits at the boundary (measured 2.1-2.8us/phase before).
        for h in range(NH):
            for ihalf in range(NHF):
                i0 = ihalf * IH
                ut_ps = utpsum.tile([128, IH], F32, tag="ut", name="ut")

                pairs = []
                quads = []
                av_queue = []
                ones_ready = []
                ones_n = [0]

                def emit_one(tl, _ut=ut_ps, _n=ones_n):
                    emit_ones(_ut, [tl], _n[0], NTILES)
                    _n[0] += 1

                def flush_pending(jt):
                    if not pending:
                        return
                    tail = pending[0]
                    avs = tail["avs"]
                    for _ in range(4):
                        if avs:
                            emit_av(tail["ut"], tail["h"], *avs.pop(0))
                    if not avs and jt >= 2:
                        for tl in tail["ones"]:
                            tail["emit_one"](tl)
                        emit_drain(tail["ut"], tail["h"], tail["half"])
                        pending.pop(0)

                expp = None
                for jt in range(NJT):
                    if jt % 2 == 0:
                        expp = streams.tile([128, 2, IH], DT_VE, tag="expst",
                                            name="expst", bufs=4)
                    s_ps = spsum.tile([128, IH], F32, tag="s", name="s_ps")
                    for q2 in range(IH // 512):
                        nc.tensor.matmul(
                            s_ps[:, q2 * 512:(q2 + 1) * 512],
                            lhsT=kpt[h][:, jt * 128:(jt + 1) * 128],
                            rhs=qpt[h][:, i0 + q2 * 512: i0 + (q2 + 1) * 512],
                            start=True, stop=True,
                        )
                    nc.scalar.activation(
                        out=expp[:, jt % 2, :], in_=s_ps,
                        func=mybir.ActivationFunctionType.Exp, scale=SCALE,
                        bias=ebias[:, 0:1],
                    )
                    flush_pending(jt)
                    if jt % 2 == 1:
                        bsbp = streams.tile([128, 2, IH], DT_VE, tag="b",
                                            name="bsb",
                                            bufs=AV_LAG // 2 + 5)
                        nc.vector.tensor_tensor(
                            out=bsbp, in0=expp,
                            in1=mt_all[:, jt - 1:jt + 1, i0:i0 + IH],
                            op=mybir.AluOpType.mult)
                        av_queue.append((jt - 1, bsbp, 0))
                        av_queue.append((jt, bsbp, 1))
                        pr = accp.tile([128, IH], DT_VE, tag="pair",
                                       name="pair", bufs=8)
                        nc.vector.tensor_tensor(
                            out=pr, in0=expp[:, 0, :], in1=expp[:, 1, :],
                            op=mybir.AluOpType.add)
                        pairs.append(pr)
                        if len(pairs) <= 2 * NQUADS:
                            if len(pairs) % 2 == 0:
                                quad = accp.tile([128, IH], DT_VE, tag="quad",
                                                 name="quad", bufs=4)
                                nc.vector.tensor_tensor(
                                    out=quad, in0=pairs[-2], in1=pairs[-1],
                                    op=mybir.AluOpType.add)
                                quads.append(quad)
                                ones_ready.append(quad)
                        else:
                            ones_ready.append(pr)
                    if not pending:
                        while len(av_queue) > AV_LAG:
                            emit_av(ut_ps, h, *av_queue.pop(0))
                        # ones-matmuls lag one tile behind formation so the
                        # PE never waits on the DVE tree mid-phase
                        while len(ones_ready) > 1:
                            emit_one(ones_ready.pop(0))

                if h == NH - 1 and ihalf == NHF - 1:
                    for jb in av_queue:
                        emit_av(ut_ps, h, *jb)
                    for tl in ones_ready:
                        emit_one(tl)
                    emit_drain(ut_ps, h, ihalf, last=True)
                else:
                    pending.append({
                        "ut": ut_ps, "h": h, "half": ihalf,
                        "avs": av_queue, "ones": ones_ready,
                        "emit_one": emit_one,
                    })
        if pending:
            tail = pending.pop(0)
            for jb in tail["avs"]:
                emit_av(tail["ut"], tail["h"], *jb)
            for tl in tail["ones"]:
                tail["emit_one"](tl)
            emit_drain(tail["ut"], tail["h"], tail["half"])

    nc.finalize()
    return nc


_NC_CACHE: dict = {}


def get_nc(NT: int = N):
    if NT not in _NC_CACHE:
        _NC_CACHE[NT] = build_nc(NT)
    return _NC_CACHE[NT]


def _pack(x):
    """[k*128, W] -> [128, k*W]: partition p holds rows {p, 128+p, ...}."""
    k = x.shape[0] // 128
    return x.reshape(k, 128, -1).transpose(1, 0, 2).reshape(128, -1)


def pack_core(qb, kb, vb, mb, wq_s, wk_s, wv_s):
    """Build one core's packed input dict from raw (transposed) slices.

    q/k/v/w go fp16 (PE-only); masks go bf16 (DVE operand).
    wq_s/wk_s: [C_in=768, NDO=384] (W.T slices); packed into 6 blocks of
    128 output dims (3 q + 3 k), block-major, ci-minor."""

    def f16(x):
        return np.ascontiguousarray(_pack(x.astype(np.float32).astype(DT_PE_NP)))

    NCT = C // 128
    wqk = np.concatenate([wq_s, wk_s], axis=1).astype(np.float32).astype(DT_PE_NP)
    P = _pack(wqk)                     # [128, NCT*2*NDO]
    blocks = []
    for b in range(NQKB):
        for ci in range(NCT):
            blocks.append(P[:, ci * 2 * NDO + b * BW: ci * 2 * NDO + (b + 1) * BW])
    wall = np.concatenate(
        blocks + [_pack(wv_s.astype(np.float32).astype(DT_PE_NP))], axis=1)
    mbf = np.ascontiguousarray(_pack(mb.astype(np.float32).astype(DT_VE_NP)))
    return {
        "qT": f16(qb), "kT": f16(kb), "vT": f16(vb), "mT": mbf,
        "wall": np.ascontiguousarray(wall),
    }


def make_in_maps(q, k, v, masks, Wq, Wk, Wv):
    """Host-side shard + layout prep. Returns per-core input dicts."""
    in_maps = []
    for c in range(N_CORES):
        b, hg = c // 2, c % 2
        in_maps.append(pack_core(
            q[b].T, k[b].T, v[b].T, masks[b].T,
            Wq[hg * NDO:(hg + 1) * NDO, :].T,
            Wk[hg * NDO:(hg + 1) * NDO, :].T,
            Wv[hg * NDV:(hg + 1) * NDV, :].T,
        ))
    return in_maps


def postprocess(res_out, masks_b, NT=N):
    """Host epilogue for one core: [65, NH*NT] raw -> [NT, NDV] slice.

    res_out rows 0..63: U^T numerator; row 64: sumexp (both carry the
    same exp bias factor, which cancels in the ratio).
    x[i, h*DV+d] = U^T[d, i] / (H * summ[i] * sumexp[i])
    """
    IH = min(1024, NT)
    NHF = NT // IH
    arr = np.asarray(res_out, np.float32).reshape(DV + 1, NH, NHF, IH)
    U = arr[:DV]                       # [DV, NH, NHF, IH]
    se = arr[DV]                       # [NH, NHF, IH]
    summ = masks_b.astype(np.float64).sum(axis=1).astype(np.float32)
    den = float(H) * summ.reshape(NHF, IH)[None, :, :] * se
    xh = U / den[None]                 # [DV, NH, NHF, IH]
    return xh.transpose(2, 3, 1, 0).reshape(NT, NH * DV)


def _reset_device():
    import ctypes
    try:
        lib = ctypes.CDLL("/opt/axon/libaxon_pjrt.so")
        lib.axon_reset.restype = ctypes.c_int64
        lib.axon_reset()
    except Exception:
        pass


def kernel(q, k, v, masks, Wq, Wk, Wv, **_unused):
    from concourse.bass_utils import run_bass_kernel_spmd

    q, k, v, masks = (np.asarray(x) for x in (q, k, v, masks))
    Wq, Wk, Wv = (np.asarray(x) for x in (Wq, Wk, Wv))

    nc = get_nc(N)
    in_maps = make_in_maps(q, k, v, masks, Wq, Wk, Wv)
    try:
        res = run_bass_kernel_spmd(
            nc, in_maps, core_ids=list(range(N_CORES))).results
    except Exception:
        # wedged accelerator (e.g. NRT_EXEC_UNIT_UNRECOVERABLE) - reset + retry
        _reset_device()
        res = run_bass_kernel_spmd(
            nc, in_maps, core_ids=list(range(N_CORES))).results

    full = np.empty((B, N, CV), np.float32)
    for c in range(N_CORES):
        b, hg = c // 2, c % 2
        full[b][:, hg * NDV:(hg + 1) * NDV] = postprocess(
            res[c]["out"], masks[b])
    return full


# revision 35
# speedup vs baseline: 1.0398x; 1.0069x over previous
"""PostCrossAttention Trainium2 kernel.

Reference computation (per batch b):
    qh = (q @ Wq.T)  split into H=8 heads of dh=96   -> [H, N, 96]
    kh = (k @ Wk.T)  likewise
    vh = (v @ Wv.T)  split into H=8 heads of dv=64   -> [H, N, 64]
    S  = qh @ kh.T * SCALE          (SCALE = (256//8)**-0.5 = 32**-0.5)
    A  = softmax(S, axis=-1)
    A  = A * m / (H * sum(m, -1, keepdims))
    x  = A @ vh   -> concat heads -> [N, 512]

Sharding: 8 cores = 4 batches x 2 head-groups (4 heads each).
Each core receives host-pre-transposed fp16 operands and computes the
un-normalized numerator U^T plus the softmax denominator row; the final
division, transpose and head-concat happen on the host.

Device dataflow (per core):
  Projections (PE at full-width M=128): Wq/Wk weights are host-packed
  into 6 blocks of 128 output dims (3 q blocks + 3 k blocks, heads
  concatenated); each block accumulates over the 6 input ci-tiles into
  4x[128,512] PSUM chunks, then is cast back out to per-head qpt/kpt
  tiles (Q casts on ACT, K casts on DVE - both idle during proj).
  Vp natural [tok, dv] as before.

  Attention (per head h, i-half of 1024; phase = 16 j-tiles):
    S.T[j,i] = Kp @ Qp.T  (PE, [128,1024] psum tiles, pool bufs=3)
    e.T = exp(S.T*SCALE - 4ln2)  (ACT, per j-tile; exps of a j-tile pair
      write the two halves of one [128,2,1024] SBUF tile)
    B.T = e.T * masks.T  (DVE, ONE [128,2,1024] op per pair - saves the
      per-op overhead vs two 1024-wide ops)
    U.T[0:64] += Vp[jt].T @ B.T[jt]  (PE, PSUM-accumulated, AV_LAG behind)
    sumexp: pair-adds (+ NQUADS quad-adds) on DVE; ones-matmuls (PE)
      contract the tiles into U.T row 64, emitted PROGRESSIVELY one tile
      behind formation (a deferred block of ones stalled ACT ~2.1us/phase)
    drain U.T -> SBUF (ACT; the attention pacer is ACT at ~18.5us/phase)
      -> DRAM; each phase's leftover A@V + last ones + drain are carried
      into the NEXT phase's jt loop so the boundary never idles ACT.

  DMA schedule (in-flight DMAs on a ring progress round-robin and finish
  TOGETHER, so priority needs staged release):
    sync ring: wq-b0, q0 (4 chunks), wq-rest, q2, q4 | gate(q4) all six
      k tiles | gate(k5) mask planes jt0-3
    scalar ring: wk, q1, q3, q5, then batches dropped into the ACT
      instruction stream at projection block boundaries (delayed issue):
      wv+v after block 2, masks jt4-9 after block 3, jt10-15 after blk 4.
    (The GpSimd software DGE ignores dependency gating - measured - so
     nothing rides it.)

Host: x[i, h*64+d] = U.T[d,i] / (8 * summ[i] * sumexp[i])   (numpy)
"""

import sys

for _p in ("/opt/trn_rl_repo",):
    if _p not in sys.path:
        sys.path.insert(0, _p)

from contextlib import ExitStack

import numpy as np

import concourse.bacc as bacc_mod
import concourse.mybir as mybir
import concourse.tile as tile

F32 = mybir.dt.float32
# PE-only operands (q/k/v/w projections) run fp16 for precision - same PE
# speed as bf16.  Everything the DVE touches (masks, exp, bsb, accumulators,
# vp) must be bf16: DVE's 2x mode engages for bf16 but NOT fp16 (measured
# 653ns vs 1030ns per [128,1024] tensor_tensor).
import ml_dtypes
DT_PE = mybir.dt.float16
DT_PE_NP = np.float16
DT_VE = mybir.dt.bfloat16
DT_VE_NP = ml_dtypes.bfloat16

# Problem constants (hardcoded per harness contract)
B, N, C, CV, H = 4, 2048, 768, 512, 8
DH, DV = C // H, CV // H          # 96, 64
NH = 4                            # heads per core
NDO = NH * DH                     # 384 projected q/k dims per core
NDV = NH * DV                     # 256 projected v dims per core
SCALE = float((256 // 8) ** (-0.5))
# shift exp into comfortable fp16 range; cancels in U/sumexp ratio
EXP_BIAS = float(-4.0 * np.log(2.0))
N_CORES = 8

NQKB = 6                          # packed q/k projection blocks of 128 dims
BW = 128

# ---- tuning knobs ----
import os
AV_LAG = int(os.environ.get("K_AV_LAG", "4"))
NQUADS = int(os.environ.get("K_NQUADS", "4"))
DRAIN_ENG = os.environ.get("K_DRAIN", "act")  # "act" | "dve" | "split"


def build_nc(NT: int = N):
    """Build the per-core Bass program. NT = token count (param for small sims)."""
    NJT = NT // 128               # j tiles
    assert NT % 512 == 0

    NCT = C // 128                # 6 c tiles
    NVT = CV // 128               # 4 cv tiles
    NCH = NT // 512               # 512-col chunks per token range
    WALL = NQKB * NCT * BW + NVT * NDV
    nc = bacc_mod.Bacc()
    # all inputs host-packed to the exact SBUF image: [128, k*W] where
    # partition p row-interleaves rows {p, 128+p, ...} of the logical tensor
    qT = nc.declare_dram_parameter("qT", [128, NCT * NT], DT_PE, isOutput=False)
    kT = nc.declare_dram_parameter("kT", [128, NCT * NT], DT_PE, isOutput=False)
    vT = nc.declare_dram_parameter("vT", [128, NVT * NT], DT_PE, isOutput=False)
    mT = nc.declare_dram_parameter("mT", [128, NJT * NT], DT_VE, isOutput=False)
    wall = nc.declare_dram_parameter("wall", [128, WALL], DT_PE, isOutput=False)
    # out rows 0..63: U^T (numerator, transposed); row 64: sumexp
    out = nc.declare_dram_parameter("out", [DV + 1, NH * NT], F32, isOutput=True)

    IH = min(1024, NT)            # i-half width
    NHF = NT // IH                # number of i-halves
    NPAIR = NJT // 2

    # q/k block -> per-head cast map: block b covers packed dims
    # [128b, 128b+128) of the NDO=384 q (or k) dims; head h owns
    # [96h, 96h+96).  Returns (psum row range, head, head-dim offset),
    # split so every piece satisfies the engine partition-window rule
    # (start 0 -> span<=128, start 64 -> <=64, start 32/96 -> <=32).
    def _pspan(s):
        return {0: 128, 32: 32, 64: 64, 96: 32}[s % 128]

    def cast_map(b):
        out_ = []
        lo, hi = BW * b, BW * b + BW
        for h in range(NH):
            hl, hh = DH * h, DH * h + DH
            a, z = max(lo, hl), min(hi, hh)
            while a < z:
                ln = min(z - a, _pspan(a - lo), _pspan(a - hl))
                out_.append((a - lo, a - lo + ln, h, a - hl))
                a += ln
        return out_

    with ExitStack() as top:
        tc = top.enter_context(tile.TileContext(nc))
        persist = top.enter_context(tc.tile_pool(name="persist", bufs=1))

        # masks (transposed) resident in SBUF
        mt_all = persist.tile([128, NJT, NT], DT_VE, tag="mt", name="mt_all")

        # ---- projections ----
        qpt = [persist.tile([DH, NT], DT_PE, tag=f"qpt{h}", name=f"qpt{h}") for h in range(NH)]
        kpt = [persist.tile([DH, NT], DT_PE, tag=f"kpt{h}", name=f"kpt{h}") for h in range(NH)]
        vp = persist.tile([128, NJT, NDV], DT_VE, tag="vp", name="vp")
        wv_sb = persist.tile([128, NVT * NDV], DT_PE, tag="wv", name="wv_sb")
        gate1 = persist.tile([128, 1], DT_PE, tag="g1", name="gate1")
        gate2 = persist.tile([128, 1], DT_PE, tag="g2", name="gate2")
        gate3 = persist.tile([128, 1], DT_PE, tag="g3", name="gate3")

        v_pool = top.enter_context(tc.tile_pool(name="vtraw", bufs=1))
        with ExitStack() as projctx:
            qkv_pool = projctx.enter_context(tc.tile_pool(name="qkv", bufs=1))
            w_pool = projctx.enter_context(tc.tile_pool(name="w", bufs=1))
            ppsum = projctx.enter_context(
                tc.tile_pool(name="ppsum", bufs=4, space="PSUM"))

            def load_tiles(dram, n_tiles, width, tag, eng, nsplit):
                t = qkv_pool.tile([128, n_tiles, width], DT_PE, tag=tag, name=tag)
                w2 = n_tiles * width
                for s in range(nsplit):
                    a, b2 = s * w2 // nsplit, (s + 1) * w2 // nsplit
                    eng.dma_start(
                        out=t.rearrange("p a n -> p (a n)")[:, a:b2],
                        in_=dram[:, a:b2])
                return t, [t[:, i, :] for i in range(n_tiles)]

            w_sb = w_pool.tile([128, WALL], DT_PE, tag="wall", name="w_sb")
            QK_END = NQKB * NCT * BW
            QB_END = QK_END // 2
            # DMA staging: in-flight DMAs on one ring progress round-robin
            # and complete TOGETHER, so priority requires staged issue.
            # Upfront (phase A, both rings): weights + all q tiles.
            # Later batches are released only as earlier data is consumed:
            # on sync via tiny gate DMAs (SP queue is idle), on scalar by
            # placing the dma_starts later in the ACT instruction stream
            # (delayed issue - the ACT queue reaches them only after the
            # corresponding cast groups, which are paced by the data).
            # sync phase A: q0's weight block first (192KB), then q-even;
            # under round-robin small DMAs complete first, so the first
            # matmul's operands (wq-b0 + q0) land earliest.
            nc.sync.dma_start(out=w_sb[:, 0:NCT * BW], in_=wall[:, 0:NCT * BW])
            nc.scalar.dma_start(out=w_sb[:, QB_END:QK_END],
                                in_=wall[:, QB_END:QK_END])

            qtile = qkv_pool.tile([128, NCT, NT], DT_PE, tag="q", name="q")
            ktile = qkv_pool.tile([128, NCT, NT], DT_PE, tag="k", name="k")
            vtile = v_pool.tile([128, NVT, NT], DT_PE, tag="v", name="v")
            qts = [qtile[:, i, :] for i in range(NCT)]
            kts = [ktile[:, i, :] for i in range(NCT)]
            vts = [vtile[:, i, :] for i in range(NVT)]
            # plain 2D slice APs so subtile deps track reliably (a
            # rearranged-AP write defeats the region matcher - measured:
            # a gate reading such a tile did not wait for its DMA)
            for hh in range(4):
                hq = NT // 4
                nc.sync.dma_start(out=qts[0][:, hh * hq:(hh + 1) * hq],
                                  in_=qT[:, hh * hq:(hh + 1) * hq])
            nc.sync.dma_start(out=w_sb[:, NCT * BW:QB_END],
                              in_=wall[:, NCT * BW:QB_END])
            for ci in (2, 4):
                nc.sync.dma_start(out=qts[ci], in_=qT[:, ci * NT:(ci + 1) * NT])
            for ci in (1, 3, 5):
                nc.scalar.dma_start(out=qts[ci], in_=qT[:, ci * NT:(ci + 1) * NT])
            # sync phase B: ALL k tiles, gated behind sync's own q batch
            # (the scalar route landed k-odd ~12us too late for the
            # ci-ordered K blocks - measured 5.5us PE gap)
            nc.sync.dma_start(out=gate1, in_=qts[NCT - 2][:, NT - 1:NT])
            for ci in range(NCT):
                nc.sync.dma_start(out=kts[ci], in_=kT[:, ci * NT:(ci + 1) * NT])
            # sync phase C: mask planes jt0-3, gated behind k
            nc.sync.dma_start(out=gate2, in_=kts[NCT - 1][:, NT - 1:NT])
            for s in range(2):
                a, b2 = 2 * s, 2 * s + 2
                nc.sync.dma_start(out=mt_all[:, a:b2, :],
                                  in_=mT[:, a * NT:b2 * NT])

            def emit_kodd_dmas():
                pass

            def emit_v_dmas():
                nc.scalar.dma_start(out=wv_sb, in_=wall[:, QK_END:WALL])
                for ci in range(NVT):
                    nc.scalar.dma_start(out=vts[ci],
                                        in_=vT[:, ci * NT:(ci + 1) * NT])

            def emit_mid_mask_dmas():
                for s in range(2, 5):
                    a, b2 = 2 * s, 2 * s + 2
                    nc.scalar.dma_start(out=mt_all[:, a:b2, :],
                                        in_=mT[:, a * NT:b2 * NT])

            def emit_late_mask_dmas():
                for s in range(5, NJT // 2):
                    a, b2 = 2 * s, 2 * s + 2
                    nc.scalar.dma_start(out=mt_all[:, a:b2, :],
                                        in_=mT[:, a * NT:b2 * NT])

            def wqk_lhsT(b, ci):
                o = (b * NCT + ci) * BW
                return w_sb[:, o:o + BW]

            # Q/K projections: 6 packed blocks (3 q then 3 k), ci-outer.
            # Delayed-issue scalar-ring DMA batches are dropped into the ACT
            # stream at block boundaries (see DMA staging note above).
            for blk_i, (t_kind, bb, xts, pts, ceng) in enumerate(
                    [("q", b, qts, qpt, nc.scalar) for b in range(3)]
                    + [("k", b, kts, kpt, nc.vector) for b in range(3)]):
                if blk_i == 1:
                    emit_v_dmas()
                elif blk_i == 3:
                    emit_mid_mask_dmas()
                elif blk_i == 4:
                    emit_late_mask_dmas()
                # psum granularity is 2 banks: pack the 4 chunk tiles as
                # 2 x [128, 2, 512]
                NCP = max(NCH // 2, 1)
                pss2 = [ppsum.tile([BW, min(NCH, 2), 512], F32, tag="pp",
                                   name="pp") for _ in range(NCP)]
                pss = [pss2[ch // 2][:, ch % 2, :] for ch in range(NCH)]
                boff = 0 if t_kind == "q" else 3
                ci_order = [c for c in range(0, NCT, 2)] + \
                           [c for c in range(1, NCT, 2)]
                for cii, ci in enumerate(ci_order):
                    for ch in range(NCH):
                        nc.tensor.matmul(
                            pss[ch],
                            lhsT=wqk_lhsT(boff + bb, ci),
                            rhs=xts[ci][:, ch * 512:(ch + 1) * 512],
                            start=(cii == 0), stop=(cii == NCT - 1),
                            skip_group_check=True,
                        )
                cm = cast_map(bb)
                for ch in range(NCH):
                    for ci_, (p0, p1, h, d0) in enumerate(cm):
                        # alternate casts between ACT and DVE (both idle
                        # during proj) so neither becomes the proj pacer
                        if ch % 2 == 0:
                            nc.scalar.copy(
                                out=pts[h][d0:d0 + (p1 - p0),
                                           ch * 512:(ch + 1) * 512],
                                in_=pss[ch][p0:p1, :])
                        else:
                            nc.vector.tensor_copy(
                                out=pts[h][d0:d0 + (p1 - p0),
                                           ch * 512:(ch + 1) * 512],
                                in_=pss[ch][p0:p1, :])


            # Vp natural: out[tok-tile, dv_all].  Four j-tiles share one
            # "pp" psum slot (4x256 fp32 regions) so ONE 1024-wide cast
            # drains them together - the per-jt cast cadence was measured
            # at ~0.9us/jt and dominated the V-proj phase.
            wvts = [wv_sb[:, i * NDV:(i + 1) * NDV] for i in range(NVT)]
            assert NJT % 4 == 0 or NJT < 4
            GV = 4 if NJT % 4 == 0 else NJT
            for g in range(NJT // GV):
                psw = ppsum.tile([BW, min(NCH, 2), 512], F32, tag="pp",
                                 name="pp")
                flat = psw.rearrange("p a b -> p (a b)")
                for k in range(GV):
                    jt = g * GV + k
                    ps = flat[:, k * NDV:(k + 1) * NDV]
                    for ci in range(NVT):
                        nc.tensor.matmul(
                            ps,
                            lhsT=vts[ci][:, jt * 128:(jt + 1) * 128],
                            rhs=wvts[ci],
                            start=(ci == 0), stop=(ci == NVT - 1),
                            skip_group_check=True,
                        )
                dst = vp[:, g * GV:(g + 1) * GV, :].rearrange("p a b -> p (a b)")
                if g % 2 == 0:
                    nc.scalar.copy(out=dst, in_=flat[:, 0:GV * NDV])
                else:
                    nc.vector.tensor_copy(out=dst, in_=flat[:, 0:GV * NDV])

        # ---- attention ----
        ones = persist.tile([128, 1], DT_VE, tag="ones", name="ones")
        nc.vector.memset(ones, 1.0)
        ebias = persist.tile([128, 1], F32, tag="ebias", name="ebias")
        nc.vector.memset(ebias, EXP_BIAS)

        spsum = top.enter_context(tc.tile_pool(name="spsum", bufs=3, space="PSUM"))
        utpsum = top.enter_context(tc.tile_pool(name="utpsum", bufs=1, space="PSUM"))
        streams = top.enter_context(tc.tile_pool(name="streams", bufs=3))
        accp = top.enter_context(tc.tile_pool(name="accp", bufs=3))
        utsb_pool = top.enter_context(tc.tile_pool(name="utsb", bufs=2))

        pending = []   # deferred drain, flushed early in next phase

        def emit_ones(ut_ps, tiles, qi0, total):
            for k, tl in enumerate(tiles):
                qi = qi0 + k
                for ic in range(IH // 512):
                    sl = slice(ic * 512, (ic + 1) * 512)
                    nc.tensor.matmul(
                        ut_ps[DV:DV + 1, sl],
                        lhsT=ones,
                        rhs=tl[:, sl],
                        start=(qi == 0), stop=(qi == total - 1),
                        skip_group_check=True,
                    )

        def emit_drain(ut_ps, h, half, last=False):
            ut_sb = utsb_pool.tile([DV + 1, IH], F32, tag="utsb", name="ut_sb")
            # ut psum -> SBUF (GPSIMD cannot read PSUM).  The final phase
            # splits the copy across ACT+DVE and the DMA four ways to
            # shorten the kernel tail.
            mode = "split" if last else DRAIN_ENG
            if mode == "act":
                nc.scalar.copy(out=ut_sb, in_=ut_ps[0:DV + 1, :])
            elif mode == "dve":
                nc.vector.tensor_copy(out=ut_sb, in_=ut_ps[0:DV + 1, :])
            else:
                hw2 = IH // 2
                nc.scalar.copy(out=ut_sb[:, 0:hw2], in_=ut_ps[0:DV + 1, 0:hw2])
                nc.vector.tensor_copy(out=ut_sb[:, hw2:IH],
                                      in_=ut_ps[0:DV + 1, hw2:IH])
            o0 = (h * NHF + half) * IH
            if last:
                # parallel dispatch on both HW queues (a sync-only 4-way
                # split serialized ~0.8us per dma_start in the tail)
                hw = IH // 2
                nc.sync.dma_start(out=out[:, o0:o0 + hw], in_=ut_sb[:, 0:hw])
                nc.scalar.dma_start(out=out[:, o0 + hw:o0 + IH],
                                    in_=ut_sb[:, hw:IH])
            else:
                hw = IH // 2
                nc.sync.dma_start(out=out[:, o0:o0 + hw], in_=ut_sb[:, 0:hw])
                nc.sync.dma_start(out=out[:, o0 + hw:o0 + IH],
                                  in_=ut_sb[:, hw:IH])

        # Per phase: 16 j-tiles processed as 8 pairs.  exp(jt) writes one
        # half of a [128,2,IH] pair tile so the mask-mult runs as a single
        # 2048-wide DVE op (saves the per-op overhead).  The exp-sum tree
        # emits its ones-matmuls PROGRESSIVELY as each pair/quad forms, so
        # at the phase boundary only the last tile's ones + the drain
        # remain - this removed a measured ~2.1us/phase ACT stall waiting
        # for a deferred block of ones-matmuls.
        NTILES = 8 - NQUADS            # ones tiles per phase

        def emit_av(ut_ps, h, jt, bsb, sub):
            first, last = (jt == 0), (jt == NJT - 1)
            for ic in range(IH // 512):
                sl = slice(ic * 512, (ic + 1) * 512)
                nc.tensor.matmul(
                    ut_ps[0:DV, sl],
                    lhsT=vp[:, jt, h * DV:(h + 1) * DV],
                    rhs=bsb[:, sub, sl],
                    start=first, stop=last, skip_group_check=True,
                )

        # pending carries the previous phase's tail INTO this phase's jt
        # loop: its leftover A@V matmuls, final ones-matmuls and the drain
        # are interleaved behind the new phase's S stream so the ACT engine
        # never waits at the boundary (measured 2.1-2.8us/phase before).
        for h in range(NH):
            for ihalf in range(NHF):
                i0 = ihalf * IH
                ut_ps = utpsum.tile([128, IH], F32, tag="ut", name="ut")

                pairs = []
                quads = []
                av_queue = []
                ones_ready = []
                ones_n = [0]

                def emit_one(tl, _ut=ut_ps, _n=ones_n):
                    emit_ones(_ut, [tl], _n[0], NTILES)
                    _n[0] += 1

                def flush_pending(jt):
                    if not pending:
                        return
                    tail = pending[0]
                    avs = tail["avs"]
                    for _ in range(4):
                        if avs:
                            emit_av(tail["ut"], tail["h"], *avs.pop(0))
                    if not avs and jt >= 2:
                        for tl in tail["ones"]:
                            tail["emit_one"](tl)
                        emit_drain(tail["ut"], tail["h"], tail["half"])
                        pending.pop(0)

                expp = None
                for jt in range(NJT):
                    if jt % 2 == 0:
                        expp = streams.tile([128, 2, IH], DT_VE, tag="expst",
                                            name="expst", bufs=4)
                    s_ps = spsum.tile([128, IH], F32, tag="s", name="s_ps")
                    for q2 in range(IH // 512):
                        nc.tensor.matmul(
                            s_ps[:, q2 * 512:(q2 + 1) * 512],
                            lhsT=kpt[h][:, jt * 128:(jt + 1) * 128],
                            rhs=qpt[h][:, i0 + q2 * 512: i0 + (q2 + 1) * 512],
                            start=True, stop=True,
                        )
                    nc.scalar.activation(
                        out=expp[:, jt % 2, :], in_=s_ps,
                        func=mybir.ActivationFunctionType.Exp, scale=SCALE,
                        bias=ebias[:, 0:1],
                    )
                    flush_pending(jt)
                    if jt % 2 == 1:
                        bsbp = streams.tile([128, 2, IH], DT_VE, tag="b",
                                            name="bsb",
                                            bufs=AV_LAG // 2 + 5)
                        nc.vector.tensor_tensor(
                            out=bsbp, in0=expp,
                            in1=mt_all[:, jt - 1:jt + 1, i0:i0 + IH],
                            op=mybir.AluOpType.mult)
                        av_queue.append((jt - 1, bsbp, 0))
                        av_queue.append((jt, bsbp, 1))
                        pr = accp.tile([128, IH], DT_VE, tag="pair",
                                       name="pair", bufs=8)
                        nc.vector.tensor_tensor(
                            out=pr, in0=expp[:, 0, :], in1=expp[:, 1, :],
                            op=mybir.AluOpType.add)
                        pairs.append(pr)
                        if len(pairs) <= 2 * NQUADS:
                            if len(pairs) % 2 == 0:
                                quad = accp.tile([128, IH], DT_VE, tag="quad",
                                                 name="quad", bufs=4)
                                nc.vector.tensor_tensor(
                                    out=quad, in0=pairs[-2], in1=pairs[-1],
                                    op=mybir.AluOpType.add)
                                quads.append(quad)
                                ones_ready.append(quad)
                        else:
                            ones_ready.append(pr)
                    if not pending:
                        while len(av_queue) > AV_LAG:
                            emit_av(ut_ps, h, *av_queue.pop(0))
                        # ones-matmuls lag one tile behind formation so the
                        # PE never waits on the DVE tree mid-phase
                        while len(ones_ready) > 1:
                            emit_one(ones_ready.pop(0))

                if h == NH - 1 and ihalf == NHF - 1:
                    for jb in av_queue:
                        emit_av(ut_ps, h, *jb)
                    for tl in ones_ready:
                        emit_one(tl)
                    emit_drain(ut_ps, h, ihalf, last=True)
                else:
                    pending.append({
                        "ut": ut_ps, "h": h, "half": ihalf,
                        "avs": av_queue, "ones": ones_ready,
                        "emit_one": emit_one,
                    })
        if pending:
            tail = pending.pop(0)
            for jb in tail["avs"]:
                emit_av(tail["ut"], tail["h"], *jb)
            for tl in tail["ones"]:
                tail["emit_one"](tl)
            emit_drain(tail["ut"], tail["h"], tail["half"])

    nc.finalize()
    return nc


_NC_CACHE: dict = {}


def get_nc(NT: int = N):
    if NT not in _NC_CACHE:
        _NC_CACHE[NT] = build_nc(NT)
    return _NC_CACHE[NT]


def _pack(x):
    """[k*128, W] -> [128, k*W]: partition p holds rows {p, 128+p, ...}."""
    k = x.shape[0] // 128
    return x.reshape(k, 128, -1).transpose(1, 0, 2).reshape(128, -1)


def pack_core(qb, kb, vb, mb, wq_s, wk_s, wv_s):
    """Build one core's packed input dict from raw (transposed) slices.

    q/k/v/w go fp16 (PE-only); masks go bf16 (DVE operand).
    wq_s/wk_s: [C_in=768, NDO=384] (W.T slices); packed into 6 blocks of
    128 output dims (3 q + 3 k), block-major, ci-minor."""

    def f16(x):
        return np.ascontiguousarray(_pack(x.astype(np.float32).astype(DT_PE_NP)))

    NCT = C // 128
    wqk = np.concatenate([wq_s, wk_s], axis=1).astype(np.float32).astype(DT_PE_NP)
    P = _pack(wqk)                     # [128, NCT*2*NDO]
    blocks = []
    for b in range(NQKB):
        for ci in range(NCT):
            blocks.append(P[:, ci * 2 * NDO + b * BW: ci * 2 * NDO + (b + 1) * BW])
    wall = np.concatenate(
        blocks + [_pack(wv_s.astype(np.float32).astype(DT_PE_NP))], axis=1)
    mbf = np.ascontiguousarray(_pack(mb.astype(np.float32).astype(DT_VE_NP)))
    return {
        "qT": f16(qb), "kT": f16(kb), "vT": f16(vb), "mT": mbf,
        "wall": np.ascontiguousarray(wall),
    }


def make_in_maps(q, k, v, masks, Wq, Wk, Wv):
    """Host-side shard + layout prep. Returns per-core input dicts."""
    in_maps = []
    for c in range(N_CORES):
        b, hg = c // 2, c % 2
        in_maps.append(pack_core(
            q[b].T, k[b].T, v[b].T, masks[b].T,
            Wq[hg * NDO:(hg + 1) * NDO, :].T,
            Wk[hg * NDO:(hg + 1) * NDO, :].T,
            Wv[hg * NDV:(hg + 1) * NDV, :].T,
        ))
    return in_maps


def postprocess(res_out, masks_b, NT=N):
    """Host epilogue for one core: [65, NH*NT] raw -> [NT, NDV] slice.

    res_out rows 0..63: U^T numerator; row 64: sumexp (both carry the
    same exp bias factor, which cancels in the ratio).
    x[i, h*DV+d] = U^T[d, i] / (H * summ[i] * sumexp[i])
    """
    IH = min(1024, NT)
    NHF = NT // IH
    arr = np.asarray(res_out, np.float32).reshape(DV + 1, NH, NHF, IH)
    U = arr[:DV]                       # [DV, NH, NHF, IH]
    se = arr[DV]                       # [NH, NHF, IH]
    summ = masks_b.astype(np.float64).sum(axis=1).astype(np.float32)
    den = float(H) * summ.reshape(NHF, IH)[None, :, :] * se
    xh = U / den[None]                 # [DV, NH, NHF, IH]
    return xh.transpose(2, 3, 1, 0).reshape(NT, NH * DV)


def _reset_device():
    import ctypes
    try:
        lib = ctypes.CDLL("/opt/axon/libaxon_pjrt.so")
        lib.axon_reset.restype = ctypes.c_int64
        lib.axon_reset()
    except Exception:
        pass


def kernel(q, k, v, masks, Wq, Wk, Wv, **_unused):
    from concourse.bass_utils import run_bass_kernel_spmd

    q, k, v, masks = (np.asarray(x) for x in (q, k, v, masks))
    Wq, Wk, Wv = (np.asarray(x) for x in (Wq, Wk, Wv))

    nc = get_nc(N)
    in_maps = make_in_maps(q, k, v, masks, Wq, Wk, Wv)
    try:
        res = run_bass_kernel_spmd(
            nc, in_maps, core_ids=list(range(N_CORES))).results
    except Exception:
        # wedged accelerator (e.g. NRT_EXEC_UNIT_UNRECOVERABLE) - reset + retry
        _reset_device()
        res = run_bass_kernel_spmd(
            nc, in_maps, core_ids=list(range(N_CORES))).results

    full = np.empty((B, N, CV), np.float32)
    for c in range(N_CORES):
        b, hg = c // 2, c % 2
        full[b][:, hg * NDV:(hg + 1) * NDV] = postprocess(
            res[c]["out"], masks[b])
    return full
